# revision 1
# baseline (speedup 1.0000x reference)
"""TRN2 Bass kernel for nn_CIFAR10_Type1_Template_Unroll (dense_cnn).

Network (per reference): two locally-connected conv layers + 3-layer FC
head, B=4096, all fp32. Strategy: pure data parallel over 8 NeuronCores
(512 batch each), activations kept on-chip in [feature, batch] layout.
Matmuls run in fp32r (full PE rate for N>=256, ~1e-4 relative error)
except L2, which runs in fp16 (~5e-4) so pairs of output positions can
execute concurrently on the PE's column strips (tile_position col
tiling is rejected by walrus for 4-byte dtypes).

Layer mapping per core (batch N=512 on the matmul free dim throughout):
- L1 (k=2,s=2 locally-connected): patches are a pure reshape of x. Host
  packs, per output row r and pair of adjacent positions, a K=32 strip
  (2 positions x 16 feats: 12 real + 4 zero-pad) and a block-diagonal
  [32, 128] weight tile. 4 pairs run concurrently on the PE's 32-row
  strips via tile_position=(32i, 0).
- L2 (k=4,s=2): positions are paired (matching the h2 tile halves);
  the two members run concurrently on col strips 0-63 / 64-127 via
  tile_position (0,0)/(0,64), each accumulating 8 K-tile matmuls
  [128K, 64M] in its own PSUM bank (start=True clears a whole bank, so
  chains must not share one).
- FC head: standard K/M tiling; FC weights host-permuted to match the
  on-chip feature order of h2 ([pos-pair, parity, channel]).
Host-side prep only reshapes/permutes weights and input (numpy).
"""
import sys

if '/opt/trn_rl_repo' not in sys.path:
    sys.path.insert(0, '/opt/trn_rl_repo')

import numpy as np

N_CORES = 8
BS = 512
LAST_EXEC_NS = None

# ----------------------------------------------------------------- host prep

def _prep_x(x):
    """x [B,3,32,32] -> [N_CORES, 16, 2, 128, 512] patch tiles.

    part = 32*i + 16*q + f; pair p=4g+i covers w1 in {2p,2p+1}; q = w1
    parity; f = c*4 + kh*2 + kw (12..15 zero-pad). Free dim = batch.
    """
    ncr = x.shape[0] // BS
    xr = x.reshape(ncr, BS, 3, 16, 2, 2, 4, 2, 2)   # s,b,c,r,kh,g,i,q,kw
    xt = xr.transpose(0, 3, 5, 6, 7, 2, 4, 8, 1)    # s,r,g,i,q,c,kh,kw,b
    xt = xt.reshape(ncr, 16, 2, 4, 2, 12, BS)
    xpp = np.zeros((ncr, 16, 2, 4, 2, 16, BS), np.float32)
    xpp[..., :12, :] = xt
    return np.ascontiguousarray(xpp.reshape(ncr, 16, 2, 128, BS))


def _prep_w1(conv1w):
    """conv1w [64,256,3,2,2] -> [16, 128, 2, 128] block-diag strips."""
    w1r = conv1w.reshape(64, 16, 16, 3, 2, 2)
    wt = w1r.transpose(1, 2, 3, 4, 5, 0).reshape(16, 16, 12, 64)
    wtp = np.zeros((16, 16, 16, 64), np.float32)
    wtp[:, :, :12, :] = wt
    wtp = wtp.reshape(16, 2, 4, 2, 16, 64)          # r,g,i,qp,f,o
    w1t = np.zeros((16, 2, 4, 2, 16, 2, 64), np.float32)
    w1t[:, :, :, 0, :, 0, :] = wtp[:, :, :, 0, :, :]
    w1t[:, :, :, 1, :, 1, :] = wtp[:, :, :, 1, :, :]
    w1t = w1t.reshape(16, 2, 128, 128).transpose(0, 2, 1, 3)
    return np.ascontiguousarray(w1t)


def _prep_w2(conv2w):
    """conv2w [64,49,64,4,4] -> [49, 128, 512] (fp16)."""
    w2r = conv2w.reshape(64, 7, 7, 64, 4, 4)
    v = w2r.transpose(1, 2, 3, 4, 5, 0)             # h,w,c,kh,kw,o
    v = v.reshape(7, 7, 64, 4, 2, 2, 64)            # h,w,c,kh,t,q,o
    v = v.transpose(0, 1, 5, 2, 3, 4, 6)            # h,w,q,c,kh,t,o
    return np.ascontiguousarray(v.reshape(49, 128, 512)).astype(np.float16)


def _h2_posmap():
    pm = np.full((25, 2), -1, np.int64)
    for T in range(21):
        rr, j = divmod(T, 3)
        pm[T, 0] = rr * 7 + 2 * j
        pm[T, 1] = rr * 7 + 2 * j + 1
    for pi in range(4):
        r0, r1 = 2 * pi, 2 * pi + 1
        pm[21 + pi, 0] = r0 * 7 + 6
        if r1 < 7:
            pm[21 + pi, 1] = r1 * 7 + 6
    return pm


def _prep_fc1(fc1):
    pm = _h2_posmap()
    fc1p = fc1.reshape(1024, 64, 49)
    fc1hat = np.zeros((1024, 25, 2, 64), np.float32)
    for T in range(25):
        for u in range(2):
            p = pm[T, u]
            if p >= 0:
                fc1hat[:, T, u, :] = fc1p[:, :, p]
    a = fc1hat.reshape(1024, 25, 128).reshape(8, 128, 25, 128)
    return np.ascontiguousarray(a.transpose(0, 3, 2, 1))   # m,kp,k,mc


def _prep_fc2(fc2):
    a = fc2.reshape(4, 128, 8, 128)
    return np.ascontiguousarray(a.transpose(0, 3, 2, 1))   # m,kp,k,mc


def _prep_fc3(fc3):
    a = fc3.T.reshape(4, 128, 10)
    return np.ascontiguousarray(a.transpose(1, 0, 2))      # kp,k,o


# --------------------------------------------------------------- bass kernel

_NC_CACHE = []


def _build_nc():
    import concourse.bass as bass
    import concourse.mybir as mybir
    from concourse import bacc
    from concourse.tile import TileContext

    f32 = mybir.dt.float32
    f32r = mybir.dt.float32r
    f16 = mybir.dt.float16
    RELU = mybir.ActivationFunctionType.Relu
    rc = lambda ap: ap

    nc = bacc.Bacc("TRN2", target_bir_lowering=False, debug=False,
                   num_devices=N_CORES)
    x_pp = nc.dram_tensor("x_pp", [16, 2, 128, BS], f32r, kind="ExternalInput")
    w1t = nc.dram_tensor("w1t", [16, 128, 2, 128], f32r, kind="ExternalInput")
    w2t = nc.dram_tensor("w2t", [49, 128, 512], f16, kind="ExternalInput")
    fc1m = nc.dram_tensor("fc1m", [8, 128, 25, 128], f32r, kind="ExternalInput")
    fc2t = nc.dram_tensor("fc2t", [4, 128, 8, 128], f32r, kind="ExternalInput")
    fc3t = nc.dram_tensor("fc3t", [128, 4, 10], f32r, kind="ExternalInput")
    zeros64 = nc.dram_tensor("zeros64", [64, 512], f32r, kind="ExternalInput")
    y = nc.dram_tensor("y", [BS, 10], f32, kind="ExternalOutput")

    pm = _h2_posmap()
    tile_of_pos = {}
    for T in range(25):
        for u in range(2):
            if pm[T, u] >= 0:
                tile_of_pos[pm[T, u]] = (T, u)

    ectr = [0]

    with TileContext(nc) as tc:
        def relu_evac(dst, src):
            if ectr[0] % 2 == 0:
                nc.scalar.activation(dst, src, RELU)
            else:
                nc.vector.tensor_scalar_max(dst, src, 0.0)
            ectr[0] += 1

        with (
            tc.tile_pool(name="h2pool", bufs=25) as h2pool,
            tc.tile_pool(name="fcw", bufs=2) as fcw_pool,
        ):
            h2 = [h2pool.tile([128, 512], f32r, tag="h2", name=f"h2_{T}")
                  for T in range(25)]
            # --------------- phase 1: L1 + L2 interleaved ---------------
            with (
                tc.tile_pool(name="xp", bufs=4) as xpp_pool,
                tc.tile_pool(name="w1p", bufs=3) as w1_pool,
                tc.tile_pool(name="w2p", bufs=6) as w2_pool,
                tc.tile_pool(name="o1p", bufs=72) as o1_pool,
                tc.tile_pool(name="l1ps", bufs=4, space="PSUM") as l1ps,
                tc.tile_pool(name="l2ps", bufs=4, space="PSUM") as l2ps,
            ):
                nc.sync.dma_start(out=h2[24][64:128, :], in_=zeros64.ap()[:])
                # PE warmup: keep the array busy during the initial DMA
                # ramp so HAM un-throttles before real matmuls arrive.
                # Dummy MMs over the (already zeroed) h2[24] hi half; the
                # consumed psum bank is start=True-cleared by later users.
                wps = l1ps.tile([128, 512], f32, tag="l1", name="warm_ps")
                zsrc = h2[24][64:96, 0:512]
                for wi in range(14):
                    nc.tensor.matmul(wps[:], zsrc[:, 0:128], zsrc[:, :],
                                     start=True, stop=True)
                out1 = [[None] * 8 for _ in range(16)]

                def emit_l1_row(r):
                    w1row = w1_pool.tile([128, 256], f32r, tag="w1",
                                         name=f"w1_{r}")
                    w1src = w1t.ap()[r].rearrange("p g c -> p (g c)")
                    if r == 0:
                        for i in range(4):
                            nc.sync.dma_start(out=w1row[32*i:32*i+32, :],
                                              in_=w1src[32*i:32*i+32, :])
                    else:
                        nc.sync.dma_start(out=w1row[:], in_=w1src)
                    for g in range(2):
                        xt = xpp_pool.tile([128, BS], f32r, tag="xp",
                                           name=f"xp_{r}_{g}")
                        if r == 0:
                            for i in range(4):
                                nc.sync.dma_start(
                                    out=xt[32*i:32*i+32, :],
                                    in_=x_pp.ap()[r, g][32*i:32*i+32, :])
                        else:
                            nc.sync.dma_start(out=xt[:], in_=x_pp.ap()[r, g])
                        for i in range(4):
                            ps = l1ps.tile([128, 512], f32, tag="l1",
                                           name=f"l1ps_{r}_{g}_{i}")
                            nc.tensor.matmul(
                                ps[:],
                                rc(w1row[32*i:32*i+32, 128*g:128*g+128]),
                                rc(xt[32*i:32*i+32, :]),
                                start=True, stop=True,
                                tile_position=(32 * i, 0))
                            ot = o1_pool.tile([128, 512], f16, tag="o1",
                                              name=f"o1_{r}_{4*g+i}")
                            relu_evac(ot[:], ps[:])
                            out1[r][4 * g + i] = ot

                def load_w2(pos):
                    w2til = w2_pool.tile([128, 512], f16, tag="w2",
                                         name=f"w2_{pos}")
                    nc.sync.dma_start(out=w2til[:], in_=w2t.ap()[pos])
                    return w2til

                def emit_l2_pair(T, hA, wA, hB, wB):
                    # Two positions concurrently on PE col strips 0-63 /
                    # 64-127 (tile_position col tiling), each chain
                    # accumulating in its own PSUM bank so the start=True
                    # bank clears stay independent of scheduler order.
                    wtA = load_w2(hA * 7 + wA)
                    wtB = None if hB is None else load_w2(hB * 7 + wB)
                    psA = l2ps.tile([128, 512], f32, tag="l2",
                                    name=f"l2psA_{T}")
                    psB = None
                    if wtB is not None:
                        psB = l2ps.tile([128, 512], f32, tag="l2",
                                        name=f"l2psB_{T}")
                    for kt in range(8):
                        kh, t = divmod(kt, 2)
                        nc.tensor.matmul(
                            psA[0:64, :],
                            wtA[:, 64*kt:64*kt+64],
                            out1[2*hA+kh][wA+t][:],
                            start=(kt == 0), stop=(kt == 7),
                            tile_position=(0, 0))
                        if wtB is not None:
                            nc.tensor.matmul(
                                psB[64:128, :],
                                wtB[:, 64*kt:64*kt+64],
                                out1[2*hB+kh][wB+t][:],
                                start=(kt == 0), stop=(kt == 7),
                                tile_position=(0, 64))
                    relu_evac(h2[T][0:64, :], psA[0:64, :])
                    if wtB is not None:
                        relu_evac(h2[T][64:128, :], psB[64:128, :])

                def emit_l2_pass(h):
                    for j in range(3):
                        emit_l2_pair(h * 3 + j, h, 2 * j, h, 2 * j + 1)
                    # cross pairs (w=6, rows h-2 & h-1) are deferred one
                    # pass: their hi-chain rhs tiles are the last evacs of
                    # row 2h-1, and the in-order PE would stall
                    # head-of-line waiting for them if emitted in pass h-1.
                    if h >= 2 and h % 2 == 0:
                        pi = (h - 2) // 2
                        emit_l2_pair(21 + pi, h - 2, 6, h - 1, 6)
                    if h == 6:
                        # Re-emitting T=23 here is intentional: it writes
                        # identical data a second time, but the extra pair
                        # keeps the PE stream dense across the last L2 pass
                        # and measures consistently faster.
                        emit_l2_pair(23, 4, 6, 5, 6)
                        emit_l2_pair(24, 6, 6, None, None)

                for r in range(16):
                    emit_l1_row(r)
                    if r == 1:
                        # second keep-warm burst: l2ps banks are idle until
                        # the first L2 pass; fills the DMA-paced early rows
                        # so HAM stays un-throttled.
                        wps2 = l2ps.tile([128, 512], f32, tag="l2",
                                         name="warm_ps2")
                        for wi in range(10):
                            nc.tensor.matmul(wps2[:], zsrc[:, 0:128],
                                             zsrc[:, :],
                                             start=True, stop=True)
                    if r >= 3 and r % 2 == 1:
                        emit_l2_pass((r - 3) // 2)

            # --------------- phase 2: FC head ---------------
            with (
                tc.tile_pool(name="fcio", bufs=12) as fcio_pool,
                tc.tile_pool(name="fcps", bufs=2, space="PSUM") as fcps,
                tc.tile_pool(name="fc3ps", bufs=2, space="PSUM") as fc3ps,
            ):
                h3 = []
                for m in range(8):
                    wt = fcw_pool.tile([128, 25 * 128], f32r, tag="fc1w",
                                       name=f"fc1w_{m}")
                    src = fc1m.ap()[m].rearrange("p k c -> p (k c)")
                    nc.sync.dma_start(out=wt[:, 0:1600], in_=src[:, 0:1600])
                    nc.sync.dma_start(out=wt[:, 1600:3200],
                                      in_=src[:, 1600:3200])
                    ps = fcps.tile([128, 512], f32, tag="fc",
                                   name=f"fc1ps_{m}")
                    for k in range(25):
                        nc.tensor.matmul(ps[:],
                                         rc(wt[:, 128*k:128*k+128]),
                                         rc(h2[k][:]),
                                         start=(k == 0), stop=(k == 24))
                    ot = fcio_pool.tile([128, 512], f32r, tag="h3",
                                        name=f"h3_{m}", bufs=8)
                    relu_evac(ot[:], ps[:])
                    h3.append(ot)
                h4 = []
                for m in range(4):
                    wt = fcw_pool.tile([128, 8 * 128], f32r, tag="fc2w",
                                       name=f"fc2w_{m}")
                    nc.sync.dma_start(
                        out=wt[:],
                        in_=fc2t.ap()[m].rearrange("p k c -> p (k c)"))
                    ps = fcps.tile([128, 512], f32, tag="fc",
                                   name=f"fc2ps_{m}")
                    for k in range(8):
                        nc.tensor.matmul(ps[:],
                                         rc(wt[:, 128*k:128*k+128]),
                                         rc(h3[k][:]),
                                         start=(k == 0), stop=(k == 7))
                    ot = fcio_pool.tile([128, 512], f32r, tag="h4",
                                        name=f"h4_{m}", bufs=4)
                    relu_evac(ot[:], ps[:])
                    h4.append(ot)
                w3 = fcio_pool.tile([128, 40], f32r, tag="fc3w",
                                    name="fc3w", bufs=1)
                nc.sync.dma_start(
                    out=w3[:], in_=fc3t.ap().rearrange("p k o -> p (k o)"))
                for b4 in range(4):
                    ps = fc3ps.tile([128, 10], f32, tag="fc3",
                                    name=f"fc3ps_{b4}")
                    for k in range(4):
                        nc.tensor.matmul(
                            ps[:],
                            rc(h4[k][:, 128*b4:128*b4+128]),
                            rc(w3[:, 10*k:10*k+10]),
                            start=(k == 0), stop=(k == 3))
                    ot = fcio_pool.tile([128, 10], f32, tag="yout",
                                        name=f"y_{b4}", bufs=4)
                    nc.vector.tensor_copy(ot[:], ps[:])
                    nc.sync.dma_start(out=y.ap()[128*b4:128*b4+128, :],
                                      in_=ot[:])
    nc.compile()
    return nc


def kernel(x, conv1w, conv2w, fc1, fc2, fc3):
    global LAST_EXEC_NS
    from concourse.bass_utils import run_bass_kernel_spmd

    x = np.ascontiguousarray(np.asarray(x, dtype=np.float32))
    conv1w = np.ascontiguousarray(np.asarray(conv1w, dtype=np.float32))
    conv2w = np.ascontiguousarray(np.asarray(conv2w, dtype=np.float32))
    fc1 = np.ascontiguousarray(np.asarray(fc1, dtype=np.float32))
    fc2 = np.ascontiguousarray(np.asarray(fc2, dtype=np.float32))
    fc3 = np.ascontiguousarray(np.asarray(fc3, dtype=np.float32))

    if not _NC_CACHE:
        _NC_CACHE.append(_build_nc())
    nc = _NC_CACHE[0]

    xpp = _prep_x(x)
    shared = {
        "zeros64": np.zeros((64, 512), np.float32),
        "w1t": _prep_w1(conv1w),
        "w2t": _prep_w2(conv2w),
        "fc1m": _prep_fc1(fc1),
        "fc2t": _prep_fc2(fc2),
        "fc3t": _prep_fc3(fc3),
    }
    in_maps = [{**shared, "x_pp": xpp[c]} for c in range(N_CORES)]
    res = run_bass_kernel_spmd(nc, in_maps, list(range(N_CORES)))
    LAST_EXEC_NS = res.exec_time_ns
    return np.concatenate([r["y"] for r in res.results], axis=0)



# revision 7
# speedup vs baseline: 1.2366x; 1.2366x over previous
"""TRN2 Bass kernel for nn_CIFAR10_Type1_Template_Unroll (dense_cnn).

Network (per reference): two locally-connected conv layers + 3-layer FC
head, B=4096. Strategy: pure data parallel over 8 NeuronCores (512 batch
each), activations kept on-chip in [feature, batch] layout, batch N=512
on the matmul free dim throughout.

v3 design notes (from baseline trace analysis):
- Everything fp16 (inputs, weights, activations; PSUM accumulate fp32).
  Measured end-to-end error ~9e-4 vs the 2e-2 gate. Halves DMA bytes.
- The PE clock is HAM-gated: 1.2GHz until ~3.4us of sustained activity,
  re-throttles on idle windows. So: full-array K=128 warmup matmuls on a
  memset tile from t~6us (no DMA dependency), and the L1/L2 emission is
  interleaved at half-row / half-pair-chain granularity so the in-order
  PE queue never head-of-line-waits on PSUM-evac completions.
- PSUM->SBUF evac runs only on ACT + DVE (GPSIMD cannot touch PSUM) at
  ~1 elem/cycle/lane, so evac INSTRUCTIONS are made as large as
  possible: L1 strips pair up in [128,1024] two-bank PSUM tiles (one
  evac per two strips), and an L2 position-pair's two chains share one
  [128,512] bank split by partition range (start=True pending-zero is
  partition-scoped), one evac per pair.
- DMA rides three independent queues: x stream on sync (q1 HWDGE),
  w1/w2/fc2/fc3 on scalar (q10 HWDGE), fc1 on gpsimd (q0 SWDGE), in
  consumption order, large transfers (per-partition lines >= 2KB).
  Buffer-reuse (WAR) hazards are resolved in emission order, so every
  pool allocation is emitted only after the previous tenant's readers.
- L1 (k=2,s=2 locally-connected): host packs per row r a K=32 strip
  (2 positions x 16 feats: 12 real + 4 zero-pad) and block-diagonal
  [32, 128] weight tiles; 4 strips run concurrently via tile_position
  row groups. L2 (k=4,s=2): positions paired on PE col strips 0-63 /
  64-127 via tile_position.
- FC3 is interleaved into the FC2 chain loop (k-major accumulation into
  4 parallel [128,10] PSUM chains) and lands in one [128,40] tile ->
  single output DMA; host undoes the [p, (b4 o)] layout.
"""
import sys

if '/opt/trn_rl_repo' not in sys.path:
    sys.path.insert(0, '/opt/trn_rl_repo')

import numpy as np

N_CORES = 8
BS = 512
WARM_N = 16
LAST_EXEC_NS = None

# ----------------------------------------------------------------- host prep

def _prep_x(x):
    """x [B,3,32,32] -> [N_CORES, 16, 128, 1024] f16 patch tiles.

    part = 32*i + 16*q + f; pair p=4g+i covers w1 in {2p,2p+1}; q = w1
    parity; f = c*4 + kh*2 + kw (12..15 zero-pad). Free dim = (g, batch).
    """
    ncr = x.shape[0] // BS
    xr = x.reshape(ncr, BS, 3, 16, 2, 2, 4, 2, 2)   # s,b,c,r,kh,g,i,q,kw
    xt = xr.transpose(0, 3, 5, 6, 7, 2, 4, 8, 1)    # s,r,g,i,q,c,kh,kw,b
    xt = xt.reshape(ncr, 16, 2, 4, 2, 12, BS)
    xpp = np.zeros((ncr, 16, 2, 4, 2, 16, BS), np.float16)
    xpp[..., :12, :] = xt
    # -> s, r, (i,q,f)=128, (g,b)=1024
    xpp = xpp.reshape(ncr, 16, 2, 128, BS).transpose(0, 1, 3, 2, 4)
    return np.ascontiguousarray(xpp.reshape(ncr, 16, 128, 1024))


def _prep_w1(conv1w):
    """conv1w [64,256,3,2,2] -> [128, 16*256] f16 block-diag strips.

    [p, r*256 + g*128 + c]: strip part p = 32i+16qp+f holds, for parity
    qp, features f -> out channel block c = 64*q + o with q==qp.
    """
    w1r = conv1w.reshape(64, 16, 16, 3, 2, 2)
    wt = w1r.transpose(1, 2, 3, 4, 5, 0).reshape(16, 16, 12, 64)
    wtp = np.zeros((16, 16, 16, 64), np.float32)
    wtp[:, :, :12, :] = wt
    wtp = wtp.reshape(16, 2, 4, 2, 16, 64)          # r,g,i,qp,f,o
    w1t = np.zeros((16, 2, 4, 2, 16, 2, 64), np.float32)
    w1t[:, :, :, 0, :, 0, :] = wtp[:, :, :, 0, :, :]
    w1t[:, :, :, 1, :, 1, :] = wtp[:, :, :, 1, :, :]
    w1t = w1t.reshape(16, 2, 128, 128)              # r,g,p,c
    w1t = w1t.transpose(2, 0, 1, 3)                 # p,r,g,c
    return np.ascontiguousarray(w1t.reshape(128, 16 * 256)).astype(np.float16)


def _h2_posmap():
    pm = np.full((25, 2), -1, np.int64)
    for T in range(21):
        rr, j = divmod(T, 3)
        pm[T, 0] = rr * 7 + 2 * j
        pm[T, 1] = rr * 7 + 2 * j + 1
    for pi in range(4):
        r0, r1 = 2 * pi, 2 * pi + 1
        pm[21 + pi, 0] = r0 * 7 + 6
        if r1 < 7:
            pm[21 + pi, 1] = r1 * 7 + 6
    return pm


# pair-tile consumption order: pass h emits pairs [3h, 3h+1, 3h+2] plus
# cross pairs 21/22/23+24 at passes 2/4/6; w2 DRAM tiles are stored in
# this exact order so each pass is one contiguous DMA.
_W2_ORDER = [0, 1, 2, 3, 4, 5, 6, 7, 8, 21, 9, 10, 11, 12, 13, 14, 22,
             15, 16, 17, 18, 19, 20, 23, 24]
_W2_SLOT = {T: s for s, T in enumerate(_W2_ORDER)}


def _prep_w2(conv2w):
    """conv2w [64,49,64,4,4] -> [25, 128, 1024] f16 pair tiles in
    consumption (_W2_ORDER) order.

    Per position: [128=(q,c), 512=(kh,t,o)]; pair tile free dim =
    (member u, 512).
    """
    w2r = conv2w.reshape(64, 7, 7, 64, 4, 4)
    v = w2r.transpose(1, 2, 3, 4, 5, 0)             # h,w,c,kh,kw,o
    v = v.reshape(7, 7, 64, 4, 2, 2, 64)            # h,w,c,kh,t,q,o
    v = v.transpose(0, 1, 5, 2, 3, 4, 6)            # h,w,q,c,kh,t,o
    pos = v.reshape(49, 128, 512)
    pm = _h2_posmap()
    out = np.zeros((25, 128, 1024), np.float16)
    for T in range(25):
        s = _W2_SLOT[T]
        out[s, :, 0:512] = pos[pm[T, 0]]
        if pm[T, 1] >= 0:
            out[s, :, 512:1024] = pos[pm[T, 1]]
    return np.ascontiguousarray(out)


def _prep_fc1(fc1):
    """fc1 [1024, 3136] -> [8, 128, 3200] f16, k in h2-tile (T) order."""
    pm = _h2_posmap()
    fc1p = fc1.reshape(1024, 64, 49)
    fc1hat = np.zeros((1024, 25, 2, 64), np.float32)
    for T in range(25):
        for u in range(2):
            p = pm[T, u]
            if p >= 0:
                fc1hat[:, T, u, :] = fc1p[:, :, p]
    a = fc1hat.reshape(8, 128, 25, 128).transpose(0, 3, 2, 1)   # m,kp,k,mc
    return np.ascontiguousarray(a.reshape(8, 128, 3200)).astype(np.float16)


def _prep_fc2(fc2):
    """fc2 [512, 1024] -> [128, 4096] f16: [kp, (m k mc)]."""
    a = fc2.reshape(4, 128, 8, 128)                 # m,mc,k,kp
    a = a.transpose(3, 0, 2, 1)                     # kp,m,k,mc
    return np.ascontiguousarray(a.reshape(128, 4096)).astype(np.float16)


def _prep_fc3(fc3):
    """fc3 [10, 512] -> [128, 40] f16: [kp, (k o)]."""
    a = fc3.T.reshape(4, 128, 10)                   # k,kp,o
    a = a.transpose(1, 0, 2)                        # kp,k,o
    return np.ascontiguousarray(a.reshape(128, 40)).astype(np.float16)


# --------------------------------------------------------------- bass kernel

_NC_CACHE = []


def _build_nc():
    import concourse.bass as bass
    import concourse.mybir as mybir
    from concourse import bacc
    from concourse.tile import TileContext

    f32 = mybir.dt.float32
    f16 = mybir.dt.float16
    RELU = mybir.ActivationFunctionType.Relu

    nc = bacc.Bacc("TRN2", target_bir_lowering=False, debug=False,
                   num_devices=N_CORES)
    x_pp = nc.dram_tensor("x_pp", [16, 128, 1024], f16, kind="ExternalInput")
    w1t = nc.dram_tensor("w1t", [128, 4096], f16, kind="ExternalInput")
    w2t = nc.dram_tensor("w2t", [25, 128, 1024], f16, kind="ExternalInput")
    fc1m = nc.dram_tensor("fc1m", [8, 128, 3200], f16, kind="ExternalInput")
    fc2t = nc.dram_tensor("fc2t", [128, 4096], f16, kind="ExternalInput")
    fc3t = nc.dram_tensor("fc3t", [128, 40], f16, kind="ExternalInput")
    y = nc.dram_tensor("y", [128, 40], f32, kind="ExternalOutput")

    pm = _h2_posmap()
    pass_pairs = {h: [3 * h + j for j in range(3)] for h in range(7)}
    pass_pairs[2].append(21)
    pass_pairs[4].append(22)
    pass_pairs[6].extend([23, 24])

    ectr = [0]

    with TileContext(nc) as tc:
        with (
            tc.tile_pool(name="h2pool", bufs=25) as h2pool,
            tc.tile_pool(name="wpool", bufs=4) as wpool,
        ):
            h2 = [h2pool.tile([128, 512], f16, tag="h2", name=f"h2_{T}")
                  for T in range(25)]

            def relu_evac(dst, src):
                if ectr[0] % 2 == 0:
                    nc.scalar.activation(dst, src, RELU)
                else:
                    nc.vector.tensor_scalar_max(dst, src, 0.0)
                ectr[0] += 1

            warm = wpool.tile([128, 512], f16, tag="warm", name="warm",
                              bufs=1)
            nc.gpsimd.memset(warm[:], 0.0)

            # fc1 weight stream (gpsimd SWDGE queue, q0)
            fc1w = [None] * 8

            def load_fc1(m):
                wt = wpool.tile([128, 3200], f16, tag="fc1w",
                                name=f"fc1w_{m}", bufs=4)
                nc.gpsimd.dma_start(out=wt[:], in_=fc1m.ap()[m])
                fc1w[m] = wt

            for m in range(4):
                load_fc1(m)

            # ---------------- phase 1: L1 + L2 interleaved ----------------
            with (
                tc.tile_pool(name="xp", bufs=6) as xp_pool,
                tc.tile_pool(name="w1p", bufs=2) as w1_pool,
                tc.tile_pool(name="w2p", bufs=3) as w2_pool,
                tc.tile_pool(name="o1p", bufs=40) as o1_pool,
                tc.tile_pool(name="l1ps", bufs=2, space="PSUM") as l1ps,
                tc.tile_pool(name="l2ps", bufs=4, space="PSUM") as l2ps,
            ):
                xt = [None] * 16

                def load_x(r):
                    t = xp_pool.tile([128, 1024], f16, tag="xp",
                                     name=f"xp_{r}")
                    nc.sync.dma_start(out=t[:], in_=x_pp.ap()[r])
                    xt[r] = t

                for r in range(4):
                    load_x(r)

                w1h = []
                for half in range(2):
                    t = w1_pool.tile([128, 2048], f16, tag="w1",
                                     name=f"w1_{half}")
                    nc.scalar.dma_start(
                        out=t[:], in_=w1t.ap()[:, 2048 * half:
                                               2048 * half + 2048])
                    w1h.append(t)

                w2tiles = {}

                def load_w2_pass(h):
                    ts = pass_pairs[h]
                    s0 = _W2_SLOT[ts[0]]
                    t = w2_pool.tile([128, 5 * 1024], f16, tag="w2",
                                     name=f"w2p_{h}")
                    src = w2t.ap()[s0:s0 + len(ts)].rearrange(
                        "t p f -> p t f")
                    dst = t[:, 0:1024 * len(ts)].rearrange(
                        "p (t f) -> p t f", t=len(ts))
                    nc.scalar.dma_start(out=dst, in_=src)
                    for j, T in enumerate(ts):
                        w2tiles[T] = t[:, 1024 * j:1024 * j + 1024]

                for h in range(3):
                    load_w2_pass(h)

                # PE warmup: full-array (K=128, M=128) matmuls on the
                # memset tile so HAM un-throttles during the DMA ramp.
                wps = l2ps.tile([128, 512], f32, tag="l2", name="warm_ps")
                for _ in range(WARM_N):
                    nc.tensor.matmul(wps[:], warm[:, 0:128], warm[:],
                                     start=True, stop=True)

                out1 = [[None] * 8 for _ in range(16)]

                def emit_l1_half(r, g):
                    w1row = w1h[r // 8][:, 256 * (r % 8):256 * (r % 8) + 256]
                    for half in range(2):
                        ps = l1ps.tile([128, 1024], f32, tag="l1",
                                       name=f"l1ps_{r}_{g}_{half}")
                        for sub in range(2):
                            i = 2 * half + sub
                            nc.tensor.matmul(
                                ps[:, 512 * sub:512 * sub + 512],
                                w1row[32 * i:32 * i + 32,
                                      128 * g:128 * g + 128],
                                xt[r][32 * i:32 * i + 32,
                                      512 * g:512 * g + 512],
                                start=True, stop=True,
                                tile_position=(32 * i, 0))
                        ot = o1_pool.tile([128, 1024], f16, tag="o1",
                                          name=f"o1_{r}_{g}_{half}")
                        relu_evac(ot[:], ps[:])
                        for sub in range(2):
                            out1[r][4 * g + 2 * half + sub] = \
                                ot[:, 512 * sub:512 * sub + 512]

                # L2 emission chunks: half-pair-chain granularity.
                chunks = []

                def push_pair(T):
                    pA, pB = pm[T]
                    hA, wA = divmod(int(pA), 7)
                    hB, wB = (None, None) if pB < 0 else divmod(int(pB), 7)
                    wt2 = w2tiles[T]
                    cell = {}

                    def steps(k0, k1):
                        def emit():
                            if k0 == 0:
                                cell['ps'] = l2ps.tile(
                                    [128, 512], f32, tag="l2",
                                    name=f"l2ps_{T}")
                            ps = cell['ps']
                            for kt in range(k0, k1):
                                kh, t = divmod(kt, 2)
                                nc.tensor.matmul(
                                    ps[0:64, :],
                                    wt2[:, 64 * kt:64 * kt + 64],
                                    out1[2 * hA + kh][wA + t],
                                    start=(kt == 0), stop=(kt == 7),
                                    tile_position=(0, 0))
                                if hB is not None:
                                    nc.tensor.matmul(
                                        ps[64:128, :],
                                        wt2[:, 512 + 64 * kt:
                                            512 + 64 * kt + 64],
                                        out1[2 * hB + kh][wB + t],
                                        start=(kt == 0), stop=(kt == 7),
                                        tile_position=(0, 64))
                            if k1 == 8:
                                relu_evac(h2[T][:], ps[:])
                        return emit
                    chunks.append(steps(0, 4))
                    chunks.append(steps(4, 8))

                cpos = [0]

                def emit_chunk():
                    if cpos[0] < len(chunks):
                        chunks[cpos[0]]()
                        cpos[0] += 1

                for r in range(16):
                    for g in range(2):
                        emit_l1_half(r, g)
                        if r >= 3:
                            emit_chunk()
                            emit_chunk()
                    if r < 12:
                        load_x(r + 4)
                    if r % 2 == 1 and r >= 3:
                        for T in pass_pairs[(r - 3) // 2]:
                            push_pair(T)
                    if r >= 5 and r % 2 == 1 and (r + 1) // 2 <= 6:
                        load_w2_pass((r + 1) // 2)
                    if r == 11:
                        fc2w = wpool.tile([128, 4096], f16, tag="fc2w",
                                          name="fc2w", bufs=1)
                        nc.scalar.dma_start(out=fc2w[:], in_=fc2t.ap())
                        fc3w = wpool.tile([128, 40], f16, tag="fc3w",
                                          name="fc3w", bufs=1)
                        nc.scalar.dma_start(out=fc3w[:], in_=fc3t.ap())
                while cpos[0] < len(chunks):
                    emit_chunk()

            # ---------------- phase 2: FC head ----------------
            with (
                tc.tile_pool(name="fcio", bufs=12) as fcio_pool,
                tc.tile_pool(name="fcps", bufs=2, space="PSUM") as fcps,
                tc.tile_pool(name="fc3ps", bufs=4, space="PSUM") as fc3ps,
            ):
                h3 = []
                for m in range(8):
                    wt = fc1w[m]
                    ps = fcps.tile([128, 512], f32, tag="fc",
                                   name=f"fc1ps_{m}")
                    for k in range(25):
                        nc.tensor.matmul(ps[:],
                                         wt[:, 128 * k:128 * k + 128],
                                         h2[k][:],
                                         start=(k == 0), stop=(k == 24))
                    ot = fcio_pool.tile([128, 512], f16, tag="h3",
                                        name=f"h3_{m}", bufs=8)
                    relu_evac(ot[:], ps[:])
                    h3.append(ot)
                    if m < 4:
                        load_fc1(m + 4)

                h4 = []
                ps3 = [fc3ps.tile([128, 10], f32, tag="fc3",
                                  name=f"fc3ps_{b4}") for b4 in range(4)]

                def emit_fc3_k(k):
                    for b4 in range(4):
                        nc.tensor.matmul(
                            ps3[b4][:],
                            h4[k][:, 128 * b4:128 * b4 + 128],
                            fc3w[:, 10 * k:10 * k + 10],
                            start=(k == 0), stop=(k == 3))

                for m in range(4):
                    ps = fcps.tile([128, 512], f32, tag="fc",
                                   name=f"fc2ps_{m}")
                    for k in range(8):
                        nc.tensor.matmul(
                            ps[:],
                            fc2w[:, 1024 * m + 128 * k:
                                 1024 * m + 128 * k + 128],
                            h3[k][:],
                            start=(k == 0), stop=(k == 7))
                    ot = fcio_pool.tile([128, 512], f16, tag="h4",
                                        name=f"h4_{m}", bufs=4)
                    relu_evac(ot[:], ps[:])
                    h4.append(ot)
                    if m >= 1:
                        emit_fc3_k(m - 1)
                emit_fc3_k(3)

                yt = fcio_pool.tile([128, 40], f32, tag="yt", name="yt",
                                    bufs=1)
                for b4 in range(4):
                    nc.vector.tensor_copy(yt[:, 10 * b4:10 * b4 + 10],
                                          ps3[b4][:])
                nc.sync.dma_start(out=y.ap()[:], in_=yt[:])
    nc.compile()
    return nc


def kernel(x, conv1w, conv2w, fc1, fc2, fc3):
    global LAST_EXEC_NS
    from concourse.bass_utils import run_bass_kernel_spmd

    x = np.ascontiguousarray(np.asarray(x, dtype=np.float32))
    conv1w = np.ascontiguousarray(np.asarray(conv1w, dtype=np.float32))
    conv2w = np.ascontiguousarray(np.asarray(conv2w, dtype=np.float32))
    fc1 = np.ascontiguousarray(np.asarray(fc1, dtype=np.float32))
    fc2 = np.ascontiguousarray(np.asarray(fc2, dtype=np.float32))
    fc3 = np.ascontiguousarray(np.asarray(fc3, dtype=np.float32))

    if not _NC_CACHE:
        _NC_CACHE.append(_build_nc())
    nc = _NC_CACHE[0]

    xpp = _prep_x(x.astype(np.float16))
    shared = {
        "w1t": _prep_w1(conv1w),
        "w2t": _prep_w2(conv2w),
        "fc1m": _prep_fc1(fc1),
        "fc2t": _prep_fc2(fc2),
        "fc3t": _prep_fc3(fc3),
    }
    in_maps = [{**shared, "x_pp": xpp[c]} for c in range(N_CORES)]
    res = run_bass_kernel_spmd(nc, in_maps, list(range(N_CORES)))
    LAST_EXEC_NS = res.exec_time_ns
    # y is [128, 40] = [p, (b4, o)] per core -> [512, 10]
    outs = []
    for r in res.results:
        yv = r["y"].reshape(128, 4, 10).transpose(1, 0, 2).reshape(512, 10)
        outs.append(yv)
    return np.ascontiguousarray(np.concatenate(outs, axis=0))
